# revision 1
# baseline (speedup 1.0000x reference)
"""Trainium2 Bass kernel for GRU(I=8,H=6) + Linear(6->4) over [B=4096, T=512].

Data-parallel over 8 NeuronCores; 512 batch rows per core, packed as 4
groups of 128 batch columns (fp16 on-device, fp32 PSUM accumulate).

The hidden state is carried as two fp16 pieces f_t = z_t*h_{t-1} and
e_t = (z_t-1)*n_t (h_t = f_t - e_t). The gates' h-side contribution for
step t+1 is accumulated by TWO small matmuls MM_f (+U @ f, off the
critical path) and MM_e (-U @ e, the only post-tanh chain op) into the
next step's PSUM bank, so h itself never sits on the loop-carried chain.
PSUM blocks per step: V@0:24 | HN@32:56 | Z@64:88 | R@96:120
(V = xn, then += u). All matmuls sit at PE tile row 0.

Loop-carried chain per step:
  MM_e (PE)   psum_{t} -= Ue' @ e_{t-1}       last h-side contribution
  sig  (ACT)  rz = sigmoid(psum[64:120]+bias) -> z@0:24, r@32:56
  u    (DVE)  u = (psum[HN@32] + b_hh_n) * r@32        fused stt
  MM_B (PE)   psum[V@0] += I @ u
  tanh (ACT)  n = tanh(psum[V@0] + b_ih_n) -> n@0:24
  e    (DVE)  e = z1 * n                                (z1 = z-1, off-path)
Off-path: MM_x (start=True), MM_f, f = z*hroll on GPSIMD, hroll = f-e on
DVE, mega h-history = f-e on GPSIMD (feeds the deferred projection),
x DMA in 16-step chunks. Output projection streams the h history through
the PE at the end (4 step-groups per PSUM bank via column tile_position),
one ACT copy per 16 steps, fp16 DMA out.
"""

import os
import sys

for _p in ("/opt/trn_rl_repo", "/root/.axon_site/_ro/trn_rl_repo"):
    if os.path.isdir(_p) and _p not in sys.path:
        sys.path.insert(0, _p)

import numpy as np

I, H, O = 8, 6, 4
B, T = 4096, 512
NCORES = 8
BS = B // NCORES        # 512 batch rows per core
G = 4                   # batch groups packed on partitions
CB = BS // G            # 128 batch columns per group
GH = G * H              # 24
GI = G * I              # 32
GO = G * O              # 16

_CACHE = {}


def _build_module():
    import concourse.tile as tile
    from concourse import bacc, mybir
    from concourse.instruction_name_ordered_set import InstructionNameOrderedSet
    from contextlib import ExitStack

    f16 = mybir.dt.float16
    f32 = mybir.dt.float32
    Sig = mybir.ActivationFunctionType.Sigmoid
    Tanh = mybir.ActivationFunctionType.Tanh
    Ident = mybir.ActivationFunctionType.Identity
    add = mybir.AluOpType.add
    mult = mybir.AluOpType.mult
    subtract = mybir.AluOpType.subtract

    nc = bacc.Bacc(
        "TRN2",
        target_bir_lowering=False,
        debug=False,
        enable_asserts=False,
        num_devices=NCORES,
    )

    NBLK = T + 1            # mega col-blocks: block t+1 holds h_t
    XCH = 16                # timesteps per x DMA chunk

    xf_d = nc.dram_tensor("xf", [GI, T * CB], f16, kind="ExternalInput").ap()
    wxf_d = nc.dram_tensor("wxf", [56, 128], f16, kind="ExternalInput").ap()
    whe_d = nc.dram_tensor("whe", [GH, 128], f16, kind="ExternalInput").ap()
    wi_d = nc.dram_tensor("wis", [GH, GH], f16, kind="ExternalInput").ap()
    wp_d = nc.dram_tensor("wps", [GH, 32], f16, kind="ExternalInput").ap()
    bias_d = nc.dram_tensor("bias", [128, 1], f32, kind="ExternalInput").ap()
    pbias_d = nc.dram_tensor("pbias", [128, 1], f32, kind="ExternalInput").ap()
    out_d = nc.dram_tensor("out", [T // 4, GO, 4 * CB], f16, kind="ExternalOutput").ap()

    with tile.TileContext(nc) as tc, ExitStack() as ctx:
        const = ctx.enter_context(tc.tile_pool(name="const", bufs=1))
        mega_pool = ctx.enter_context(tc.tile_pool(name="mega", bufs=1))
        xpool = ctx.enter_context(tc.tile_pool(name="x", bufs=3))
        ps_pool = ctx.enter_context(tc.tile_pool(name="ps", bufs=3, space="PSUM"))
        rz_pool = ctx.enter_context(tc.tile_pool(name="rz", bufs=6))
        n_pool = ctx.enter_context(tc.tile_pool(name="n", bufs=6))
        u_pool = ctx.enter_context(tc.tile_pool(name="u", bufs=6))
        e_pool = ctx.enter_context(tc.tile_pool(name="e", bufs=6))
        fl_pool = ctx.enter_context(tc.tile_pool(name="fl", bufs=1))
        pp_pool = ctx.enter_context(tc.tile_pool(name="pp", bufs=2, space="PSUM"))
        ob_pool = ctx.enter_context(tc.tile_pool(name="ob", bufs=2))

        wxf_s = const.tile([56, 128], f16)
        nc.gpsimd.dma_start(wxf_s[:], wxf_d)
        whe_s = const.tile([56, 128], f16)
        nc.scalar.dma_start(whe_s[32:56, :], whe_d)
        wi_s = const.tile([GH, GH], f16)
        nc.gpsimd.dma_start(wi_s[:], wi_d)
        wp_s = const.tile([GH, 32], f16)
        nc.scalar.dma_start(wp_s[:], wp_d)
        bias_s = const.tile([128, 1], f32)
        nc.scalar.dma_start(bias_s[:], bias_d)
        pbias_s = const.tile([128, 1], f32)
        nc.gpsimd.dma_start(pbias_s[:], pbias_d)

        mega = mega_pool.tile([GH, NBLK * CB], f16)

        def blk(t):
            return slice(t * CB, (t + 1) * CB)

        xtiles = {}

        def fetch_chunk(c):
            if c * XCH >= T or c in xtiles:
                return
            xt = xpool.tile([56, XCH * CB], f16, name="xt", tag="xt")
            nc.sync.dma_start(
                xt[0:GI, :], xf_d[:, c * XCH * CB : (c + 1) * XCH * CB]
            )
            xtiles[c] = xt

        fetch_chunk(0)
        fetch_chunk(1)
        nc.vector.memset(mega[:, 0:CB], 0.0)    # h_{-1} = 0

        def mm_x(t, ps):
            c, s = t // XCH, t % XCH
            xt = xtiles[c]
            nc.tensor.matmul(
                ps[0:128, :],
                wx_s[:],
                xt[:, s * CB : (s + 1) * CB],
                start=True,
                stop=False,
            )

        HB = CB // 2            # half-batch columns per chain

        def new_ps(h):
            return ps_pool.tile([128, HB], f32, name=f"ps{h}", tag=f"ps{h}")

        def hcols(x0):
            return slice(x0, x0 + HB)

        def mm_xf(t, h, ps, after):
            c, s = t // XCH, t % XCH
            xt = xtiles[c]
            mm = nc.tensor.matmul(
                ps[0:128, :],
                wxf_s[:],
                xt[0:56, s * CB + h * HB : s * CB + (h + 1) * HB],
                start=True,
                stop=False,
            )
            if after is not None:
                deps = InstructionNameOrderedSet()
                deps.add(after.ins.name)
                mm.ins.add_nosync_dependencies_from(deps)
            return mm

        nc.vector.memset(xtiles[0][32:56, 0:CB], 0.0)   # f_{-1} = 0

        ps_cur = [new_ps(0), new_ps(1)]
        mm_xf(0, 0, ps_cur[0], None)
        mm_xf(0, 1, ps_cur[1], None)

        PLAG = 16               # proj runs PLAG steps behind the chain
        pp_cur = {}
        ob_cur = {}

        def proj_step(tp, after, act_after=None):
            """Emit the projection matmul for step tp (reads mega block
            tp+1 = h_tp); one [24,128] matmul into the chunk's psum bank."""
            c, s = tp // XCH, tp % XCH
            j, col = s // 4, s % 4
            if s == 0:
                pp_cur[c] = pp_pool.tile([128, 4 * CB], f32, name="pp", tag="pp")
            pp = pp_cur[c]
            mm = nc.tensor.matmul(
                pp[32 * j : 32 * j + 32, col * CB : (col + 1) * CB],
                wp_s[:],
                mega[:, (tp + 1) * CB : (tp + 2) * CB],
                start=(col == 0), stop=(col == 3),
                tile_position=(0, 32 * j),
            )
            if after is not None:
                deps = InstructionNameOrderedSet()
                deps.add(after.ins.name)
                mm.ins.add_nosync_dependencies_from(deps)
            if s >= XCH - 4:
                # column col2 of the chunk's psum is complete at s=12+col2;
                # copy it out in four small pieces so no single ACT op
                # blocks the chain's sigmoid/tanh at chunk boundaries.
                col2 = s - (XCH - 4)
                if col2 == 0:
                    ob_cur[c] = ob_pool.tile(
                        [128, 4 * CB], f16, name="ob", tag="ob"
                    )
                ob = ob_cur[c]
                oc = nc.scalar.activation(
                    ob[:, col2 * CB : (col2 + 1) * CB],
                    pp[:, col2 * CB : (col2 + 1) * CB],
                    Ident, bias=pbias_s[:],
                )
                if act_after is not None:
                    deps2 = InstructionNameOrderedSet()
                    deps2.add(act_after.ins.name)
                    oc.ins.add_nosync_dependencies_from(deps2)
            if s == XCH - 1:
                ob = ob_cur[c]
                for jj in range(4):
                    nc.sync.dma_start(
                        out_d[c * 4 + jj, :, :], ob[32 * jj : 32 * jj + GO, :]
                    )
                del pp_cur[c]
                del ob_cur[c]
            return mm

        prev_tanh = None
        for t in range(T):
            if t % XCH == 8:
                fetch_chunk(t // XCH + 2)

            rz = [None, None]
            u_t = [None, None]
            mmb = [None, None]
            e_t = [None, None]
            n_t = [None, None]
            tnh = [None, None]
            ps_next = [None, None]

            for h in range(2):
                # combined r+z sigmoid: psum Z@64->z@0:24, R@96->r@32:56
                rz[h] = rz_pool.tile([56, HB], f16, name=f"rz{h}", tag=f"rz{h}")
                nc.scalar.activation(
                    rz[h][0:56, :], ps_cur[h][64:120, :], Sig,
                    bias=bias_s[64:120],
                )

            for h in range(2):
                u_t[h] = u_pool.tile([GH, HB], f16, name=f"u{h}", tag=f"u{h}")
                nc.vector.scalar_tensor_tensor(
                    out=u_t[h][:], in0=ps_cur[h][32:56, :],
                    scalar=bias_s[32:56], in1=rz[h][32:56, :],
                    op0=add, op1=mult,
                )

            for h in range(2):
                mmb[h] = nc.tensor.matmul(
                    ps_cur[h][0:24, :], wi_s[:], u_t[h][:],
                    start=False, stop=True,
                )

            pmm = (proj_step(t - PLAG, mmb[1], act_after=prev_tanh)
                   if t >= PLAG else None)

            # f = z * h_{t-1} on GPSIMD -> x-tile rows 32:56 of step t+1's
            # column (feeds MM_xf); off the critical path.
            if t + 1 < T:
                c1, s1 = (t + 1) // XCH, (t + 1) % XCH
                fdst = [
                    xtiles[c1][32:56, s1 * CB + h * HB : s1 * CB + (h + 1) * HB]
                    for h in range(2)
                ]
            else:
                flast = fl_pool.tile([56, CB], f16, name="flast", tag="fl")
                fdst = [flast[32:56, hcols(h * HB)] for h in range(2)]
            for h in range(2):
                nc.gpsimd.tensor_tensor(
                    out=fdst[h], in0=rz[h][0:24, :],
                    in1=mega[:, blk(t)][:, hcols(h * HB)], op=mult,
                )

            for h in range(2):
                n_t[h] = n_pool.tile([GH, HB], f16, name=f"n{h}", tag=f"n{h}")
                tnh[h] = nc.scalar.activation(
                    n_t[h][:], ps_cur[h][0:24, :], Tanh, bias=bias_s[0:24]
                )

                # e = (z - 1) * n  (chain) -> feeds MM_e into next bank
                e_t[h] = e_pool.tile([56, HB], f16, name=f"e{h}", tag=f"e{h}")
                nc.vector.scalar_tensor_tensor(
                    out=e_t[h][32:56, :], in0=rz[h][0:24, :], scalar=1.0,
                    in1=n_t[h][:], op0=subtract, op1=mult,
                )

            if t + 1 < T:
                for h in range(2):
                    ps_next[h] = new_ps(h)
                    mmxf = mm_xf(t + 1, h, ps_next[h],
                                 pmm if (h == 0 and pmm is not None) else mmb[h])
                    # e-side contribution (the chain's last link into t+1)
                    mme = nc.tensor.matmul(
                        ps_next[h][0:128, :], whe_s[32:56, :], e_t[h][32:56, :],
                        start=False, stop=False,
                    )
                    deps = InstructionNameOrderedSet()
                    deps.add(mmxf.ins.name)
                    mme.ins.add_nosync_dependencies_from(deps)

            for h in range(2):
                # h_t = f - e -> mega (h history + next step's f input)
                nc.gpsimd.tensor_tensor(
                    out=mega[:, blk(t + 1)][:, hcols(h * HB)],
                    in0=fdst[h], in1=e_t[h][32:56, :], op=subtract,
                )

            prev_tanh = tnh[1]
            ps_cur = ps_next

        # flush the projection for the last PLAG steps
        for tp in range(T - PLAG, T):
            proj_step(tp, None)

    nc.compile()
    return nc


def _pack_weights(W_ih, W_hh, b_ih, b_hh, W_lin, b_lin):
    # PSUM blocks: V@0:24 | HN@32:56 | Z@64:88 | R@96:120
    # wxf rows 0:32 = x weights, rows 32:56 = +U (f rows); whe = -U (e rows)
    wxf = np.zeros((56, 128), np.float32)
    whe = np.zeros((GH, 128), np.float32)
    wp = np.zeros((GH, 32), np.float32)
    bias = np.zeros((128, 1), np.float32)
    pbias = np.zeros((128, 1), np.float32)
    Ur, Uz, Un = W_hh[0:6], W_hh[6:12], W_hh[12:18]
    Wr, Wz, Wn = W_ih[0:6], W_ih[6:12], W_ih[12:18]
    for g in range(G):
        hsl = slice(g * H, (g + 1) * H)
        xsl = slice(g * I, (g + 1) * I)
        wxf[xsl, 0 + g * H : 6 + g * H] = Wn.T    # V = xn
        wxf[xsl, 64 + g * H : 70 + g * H] = Wz.T
        wxf[xsl, 96 + g * H : 102 + g * H] = Wr.T
        fsl = slice(32 + g * H, 38 + g * H)
        wxf[fsl, 32 + g * H : 38 + g * H] = Un.T
        wxf[fsl, 64 + g * H : 70 + g * H] = Uz.T
        wxf[fsl, 96 + g * H : 102 + g * H] = Ur.T
        whe[hsl, 32 + g * H : 38 + g * H] = -Un.T
        whe[hsl, 64 + g * H : 70 + g * H] = -Uz.T
        whe[hsl, 96 + g * H : 102 + g * H] = -Ur.T
        wp[hsl, g * O : (g + 1) * O] = W_lin.T
        bias[0 + g * H : 6 + g * H, 0] = b_ih[12:18]            # tanh V bias
        bias[32 + g * H : 38 + g * H, 0] = b_hh[12:18]          # u stt scalar
        bias[64 + g * H : 70 + g * H, 0] = b_ih[6:12] + b_hh[6:12]   # z
        bias[96 + g * H : 102 + g * H, 0] = b_ih[0:6] + b_hh[0:6]    # r
        for j in range(4):
            pbias[32 * j + g * O : 32 * j + (g + 1) * O, 0] = b_lin
    wi = np.eye(GH, dtype=np.float32)
    return (
        wxf.astype(np.float16),
        whe.astype(np.float16),
        wi.astype(np.float16),
        wp.astype(np.float16),
        bias,
        pbias,
    )


def _run(inputs, trace=False):
    from concourse.bass_utils import run_bass_kernel_spmd

    x = np.asarray(inputs["x"], dtype=np.float32)
    W_ih = np.asarray(inputs["W_ih"], np.float32)
    W_hh = np.asarray(inputs["W_hh"], np.float32)
    b_ih = np.asarray(inputs["b_ih"], np.float32)
    b_hh = np.asarray(inputs["b_hh"], np.float32)
    W_lin = np.asarray(inputs["W_lin"], np.float32)
    b_lin = np.asarray(inputs["b_lin"], np.float32)

    if "nc" not in _CACHE:
        _CACHE["nc"] = _build_module()
    nc = _CACHE["nc"]

    wxf, whe, wi, wp, bias, pbias = _pack_weights(
        W_ih, W_hh, b_ih, b_hh, W_lin, b_lin
    )

    in_maps = []
    for c in range(NCORES):
        xc = x[c * BS : (c + 1) * BS]                    # [512, 512, 8]
        xf = (
            xc.reshape(G, CB, T, I)
            .transpose(0, 3, 2, 1)                       # [g, i, t, b]
            .reshape(GI, T * CB)
            .astype(np.float16)
        )
        in_maps.append(
            {"xf": xf, "wxf": wxf, "whe": whe, "wis": wi,
             "wps": wp, "bias": bias, "pbias": pbias}
        )

    res = run_bass_kernel_spmd(
        nc, in_maps, core_ids=list(range(NCORES)), trace=trace
    )

    outs = []
    for c in range(NCORES):
        a = res.results[c]["out"].astype(np.float32)     # [T/4, 16, 512]
        a = a.reshape(T // 4, G, O, 4, CB)               # [t4, g, o, tt, b]
        a = a.transpose(1, 4, 0, 3, 2)                   # [g, b, t4, tt, o]
        outs.append(a.reshape(BS, T, O))
    full = np.concatenate(outs, axis=0)
    return full, res


def kernel(**inputs) -> np.ndarray:
    out, _ = _run(inputs, trace=False)
    return out


def kernel_profiled(inputs):
    """Returns (output, BassKernelResults-with-trace)."""
    return _run(inputs, trace=True)

